# revision 4
# baseline (speedup 1.0000x reference)
"""Bayesian-router MoE kernel for 8 Trainium2 NeuronCores.

Strategy (expert-parallel, per sharding hint):
  - Router moments / top-k / combine weights: tiny (B*F*E ~ 17 MFLOP), computed
    on host in float64 (min rank4/rank5 score gap is ~1.7e-4, far above fp32
    noise, so expert selection is stable vs the fp32 reference).
  - Token dispatch: host gathers each expert's routed tokens into a padded,
    transposed buffer XgT [F, CAP] (the host-side equivalent of the
    all-to-all; full I/O contract means shard/unshard happens on host).
  - Device: each of the 8 cores runs the 2-expert MLP on its gathered tokens,
    entirely in transposed form (A1T = relu(W1^T XgT + b1), YT = W2^T A1T + b2)
    so no on-device transposes are needed; weights stream as lhsT directly.
  - Combine: host scatter-adds w[t,e] * Y_e rows into the output (the
    cross-device reduction of the unshard step).
"""

import os
import numpy as np

NCORES = 8
P = 128
TOP_K = 4


# ---------------------------------------------------------------------------
# host-side routing (matches reference math; float64 for stable ordering)
# ---------------------------------------------------------------------------
def _routing(h, W_mu, b_mu, W_logvar, b_logvar):
    h64 = h.astype(np.float64)
    mu = h64 @ W_mu.T.astype(np.float64) + b_mu.astype(np.float64)
    var = (h64 * h64) @ np.exp(W_logvar.astype(np.float64)).T + np.exp(
        b_logvar.astype(np.float64)
    )
    var = np.maximum(var, 1e-12)
    tilde = mu / np.sqrt(1.0 + (np.pi / 8.0) * var)
    t = tilde - tilde.max(axis=1, keepdims=True)
    ex = np.exp(t)
    probs = ex / ex.sum(axis=1, keepdims=True)
    idx = np.argsort(-tilde, axis=1, kind="stable")[:, :TOP_K]
    w = np.take_along_axis(probs, idx, axis=1)
    w = w / np.maximum(w.sum(axis=1, keepdims=True), 1e-12)
    return idx, w


# ---------------------------------------------------------------------------
# device kernel: 2-expert MLP on pre-gathered transposed tokens
# ---------------------------------------------------------------------------
def _build_kernel(epc, F, H, C, cap, chunks):
    import concourse.mybir as mybir
    import concourse.tile as tile
    from concourse import bacc

    f32 = mybir.dt.float32
    f32r = mybir.dt.float32r
    FK, HK, CK = F // P, H // P, C // P

    nc = bacc.Bacc("TRN2", target_bir_lowering=False, debug=False,
                   num_devices=NCORES)

    # inputs feeding fp32r matmuls are declared float32r end-to-end (same
    # bytes as fp32) so plain HWDGE DMAs satisfy the BIR fp32r-rounding rule
    xt = nc.dram_tensor("xt", [epc, F, cap], f32r, kind="ExternalInput")
    w1 = nc.dram_tensor("w1", [epc, F, H], f32r, kind="ExternalInput")
    w2 = nc.dram_tensor("w2", [epc, H, C], f32r, kind="ExternalInput")
    b1 = nc.dram_tensor("b1", [P, epc, HK], f32, kind="ExternalInput")
    b2 = nc.dram_tensor("b2", [P, epc, CK], f32, kind="ExternalInput")
    yt = nc.dram_tensor("yt", [epc, C, cap], f32, kind="ExternalOutput")

    with tile.TileContext(nc) as tc:
        with (
            tc.tile_pool(name="consts", bufs=1) as consts,
            tc.tile_pool(name="w1pool", bufs=2) as w1pool,
            tc.tile_pool(name="w2pool", bufs=2) as w2pool,
            tc.tile_pool(name="xpool", bufs=2) as xpool,
            tc.tile_pool(name="apool", bufs=2) as apool,
            tc.tile_pool(name="ypool", bufs=2) as ypool,
            tc.tile_pool(name="psum", bufs=4, space="PSUM") as pp,
        ):
            b1s = consts.tile([P, epc, HK], f32)
            nc.gpsimd.dma_start(out=b1s[:], in_=b1[:])
            b2s = consts.tile([P, epc, CK], f32)
            nc.gpsimd.dma_start(out=b2s[:], in_=b2[:])

            for e in range(epc):
                # chunked input DMAs (per k-chunk) on the two HWDGE rings so
                # the first matmuls start as soon as (xt k0, w1 k0) land
                xts = xpool.tile([P, FK, cap], f32r, tag="xt")
                xt_r = xt[e].rearrange("(k p) n -> p k n", p=P)
                w1s = w1pool.tile([P, FK, H], f32r, tag="w1")
                w1_r = w1[e].rearrange("(k p) m -> p k m", p=P)
                for k in range(FK):
                    nc.sync.dma_start(out=xts[:, k], in_=xt_r[:, k])
                    nc.scalar.dma_start(out=w1s[:, k], in_=w1_r[:, k])
                w2s = w2pool.tile([P, HK, C], f32r, tag="w2")
                w2_r = w2[e].rearrange("(k p) m -> p k m", p=P)
                for k in range(0, HK, 2):
                    nc.scalar.dma_start(
                        out=w2s[:, k:k + 2], in_=w2_r[:, k:k + 2]
                    )

                a1s = apool.tile([P, HK, cap], f32r, tag="a1")
                yts = ypool.tile([P, CK, cap], f32, tag="yt")
                yt_r = yt[e].rearrange("(k p) n -> p k n", p=P)

                for n0, nsz in chunks:
                    # layer 1: A1T[m] = relu(sum_k W1[k,m]^T @ XgT[k] + b1[m])
                    for m in range(HK):
                        ps = pp.tile([P, 512], f32, tag="ps")
                        for k in range(FK):
                            nc.tensor.matmul(
                                ps[:, :nsz],
                                w1s[:, k, m * P:(m + 1) * P],
                                xts[:, k, n0:n0 + nsz],
                                start=(k == 0),
                                stop=(k == FK - 1),
                            )
                        nc.scalar.activation(
                            a1s[:, m, n0:n0 + nsz],
                            ps[:, :nsz],
                            mybir.ActivationFunctionType.Relu,
                            bias=b1s[:, e, m:m + 1],
                        )
                    # layer 2: YT[m] = sum_k W2[k,m]^T @ A1T[k] + b2[m]
                    for m in range(CK):
                        ps = pp.tile([P, 512], f32, tag="ps")
                        for k in range(HK):
                            nc.tensor.matmul(
                                ps[:, :nsz],
                                w2s[:, k, m * P:(m + 1) * P],
                                a1s[:, k, n0:n0 + nsz],
                                start=(k == 0),
                                stop=(k == HK - 1),
                            )
                        nc.vector.tensor_scalar_add(
                            yts[:, m, n0:n0 + nsz],
                            ps[:, :nsz],
                            b2s[:, e, m:m + 1],
                        )
                        if n0 + nsz == cap:
                            # whole row of yts done -> stream it out
                            nc.sync.dma_start(out=yt_r[:, m], in_=yts[:, m])

    nc.compile()
    return nc


def _chunks(cap):
    n = (cap + 511) // 512
    base, rem = divmod(cap, n)
    out = []
    off = 0
    for i in range(n):
        sz = base + (1 if i < rem else 0)
        out.append((off, sz))
        off += sz
    return out


# ---------------------------------------------------------------------------
# entry point
# ---------------------------------------------------------------------------
def kernel(h, W_mu, b_mu, W_logvar, b_logvar, W1, b1, W2, b2):
    from concourse.bass_utils import run_bass_kernel_spmd

    h = np.ascontiguousarray(np.asarray(h, dtype=np.float32))
    W1 = np.asarray(W1, dtype=np.float32)
    b1 = np.asarray(b1, dtype=np.float32)
    W2 = np.asarray(W2, dtype=np.float32)
    b2 = np.asarray(b2, dtype=np.float32)

    B, F = h.shape
    E, _, H = W1.shape
    C = W2.shape[2]
    assert E % NCORES == 0
    epc = E // NCORES
    FK, HK, CK = F // P, H // P, C // P

    topk_idx, topk_w = _routing(
        np.asarray(h), np.asarray(W_mu), np.asarray(b_mu),
        np.asarray(W_logvar), np.asarray(b_logvar)
    )

    # per-expert token lists
    toks, poss = [], []
    for e in range(E):
        tok, pos = np.nonzero(topk_idx == e)
        toks.append(tok)
        poss.append(pos)
    maxcnt = max(len(t) for t in toks)
    cap = max(64, -(-maxcnt // 32) * 32)
    chunks = _chunks(cap)

    # gather/dispatch: XgT per expert, padded to cap
    xt = np.zeros((NCORES, epc, F, cap), np.float32)
    for e in range(E):
        cnt = len(toks[e])
        xt[e // epc, e % epc, :, :cnt] = h[toks[e]].T

    # biases pre-swizzled to [P, epc, K] so the device DMA is contiguous
    b1_sw = np.ascontiguousarray(
        b1.reshape(E, HK, P).transpose(2, 0, 1).reshape(P, NCORES, epc, HK)
        .transpose(1, 0, 2, 3)
    )  # [NCORES, P, epc, HK]
    b2_sw = np.ascontiguousarray(
        b2.reshape(E, CK, P).transpose(2, 0, 1).reshape(P, NCORES, epc, CK)
        .transpose(1, 0, 2, 3)
    )

    nc = _build_kernel(epc, F, H, C, cap, chunks)

    in_maps = []
    for c in range(NCORES):
        lo, hi = c * epc, (c + 1) * epc
        in_maps.append({
            "xt": xt[c],
            "w1": W1[lo:hi],
            "w2": W2[lo:hi],
            "b1": b1_sw[c],
            "b2": b2_sw[c],
        })

    trace = bool(os.environ.get("MOE_KERNEL_TRACE"))
    res = run_bass_kernel_spmd(nc, in_maps, list(range(NCORES)), trace=trace)
    global LAST_RESULTS
    LAST_RESULTS = res

    # combine: scatter-add weighted expert outputs
    out = np.zeros((B, C), np.float32)
    for e in range(E):
        cnt = len(toks[e])
        yte = res.results[e // epc]["yt"][e % epc]  # [C, cap]
        out[toks[e]] += (
            topk_w[toks[e], poss[e]].astype(np.float32)[:, None]
            * yte[:, :cnt].T
        )
    return out


LAST_RESULTS = None


# revision 5
# speedup vs baseline: 1.0116x; 1.0116x over previous
"""Bayesian-router MoE kernel for 8 Trainium2 NeuronCores.

Strategy (expert-parallel, per sharding hint):
  - Router moments / top-k / combine weights: tiny (B*F*E ~ 17 MFLOP), computed
    on host in float64 (min rank4/rank5 score gap is ~1.7e-4, far above fp32
    noise, so expert selection is stable vs the fp32 reference).
  - Token dispatch: host gathers each expert's routed tokens into a padded,
    transposed buffer XgT [F, CAP] (the host-side equivalent of the
    all-to-all; full I/O contract means shard/unshard happens on host).
  - Device: each of the 8 cores runs the 2-expert MLP on its gathered tokens,
    entirely in transposed form (A1T = relu(W1^T XgT + b1), YT = W2^T A1T + b2)
    so no on-device transposes are needed; weights stream as lhsT directly.
  - Combine: host scatter-adds w[t,e] * Y_e rows into the output (the
    cross-device reduction of the unshard step).
"""

import os
import numpy as np

NCORES = 8
P = 128
TOP_K = 4


# ---------------------------------------------------------------------------
# host-side routing (matches reference math; float64 for stable ordering)
# ---------------------------------------------------------------------------
def _routing(h, W_mu, b_mu, W_logvar, b_logvar):
    h64 = h.astype(np.float64)
    mu = h64 @ W_mu.T.astype(np.float64) + b_mu.astype(np.float64)
    var = (h64 * h64) @ np.exp(W_logvar.astype(np.float64)).T + np.exp(
        b_logvar.astype(np.float64)
    )
    var = np.maximum(var, 1e-12)
    tilde = mu / np.sqrt(1.0 + (np.pi / 8.0) * var)
    t = tilde - tilde.max(axis=1, keepdims=True)
    ex = np.exp(t)
    probs = ex / ex.sum(axis=1, keepdims=True)
    idx = np.argsort(-tilde, axis=1, kind="stable")[:, :TOP_K]
    w = np.take_along_axis(probs, idx, axis=1)
    w = w / np.maximum(w.sum(axis=1, keepdims=True), 1e-12)
    return idx, w


# ---------------------------------------------------------------------------
# device kernel: 2-expert MLP on pre-gathered transposed tokens
# ---------------------------------------------------------------------------
def _build_kernel(epc, F, H, C, cap, chunks):
    import concourse.mybir as mybir
    import concourse.tile as tile
    from concourse import bacc

    f32 = mybir.dt.float32
    f32r = mybir.dt.float32r
    FK, HK, CK = F // P, H // P, C // P

    nc = bacc.Bacc("TRN2", target_bir_lowering=False, debug=False,
                   num_devices=NCORES)

    # inputs feeding fp32r matmuls are declared float32r end-to-end (same
    # bytes as fp32) so plain HWDGE DMAs satisfy the BIR fp32r-rounding rule
    xt = nc.dram_tensor("xt", [epc, F, cap], f32r, kind="ExternalInput")
    w1 = nc.dram_tensor("w1", [epc, F, H], f32r, kind="ExternalInput")
    w2 = nc.dram_tensor("w2", [epc, H, C], f32r, kind="ExternalInput")
    b1 = nc.dram_tensor("b1", [P, epc, HK], f32, kind="ExternalInput")
    b2 = nc.dram_tensor("b2", [P, epc, CK], f32, kind="ExternalInput")
    yt = nc.dram_tensor("yt", [epc, C, cap], f32, kind="ExternalOutput")

    with tile.TileContext(nc) as tc:
        with (
            tc.tile_pool(name="consts", bufs=1) as consts,
            tc.tile_pool(name="w1pool", bufs=2) as w1pool,
            tc.tile_pool(name="w2pool", bufs=2) as w2pool,
            tc.tile_pool(name="xpool", bufs=2) as xpool,
            tc.tile_pool(name="apool", bufs=2) as apool,
            tc.tile_pool(name="ypool", bufs=2) as ypool,
            tc.tile_pool(name="psum", bufs=4, space="PSUM") as pp,
        ):
            b1s = consts.tile([P, epc, HK], f32)
            nc.gpsimd.dma_start(out=b1s[:], in_=b1[:])
            b2s = consts.tile([P, epc, CK], f32)
            nc.gpsimd.dma_start(out=b2s[:], in_=b2[:])

            add, amax = mybir.AluOpType.add, mybir.AluOpType.max

            def evict(i, dst, src, bias, relu):
                # alternate PSUM evictions between Scalar(ACT) and Vector(DVE)
                # so neither engine falls behind the matmul stream
                if i % 2 == 0:
                    nc.scalar.activation(
                        dst, src,
                        mybir.ActivationFunctionType.Relu if relu
                        else mybir.ActivationFunctionType.Identity,
                        bias=bias,
                    )
                elif relu:
                    nc.vector.tensor_scalar(dst, src, bias, 0.0, add, amax)
                else:
                    nc.vector.tensor_scalar_add(dst, src, bias)

            for e in range(epc):
                # token chunks on one HWDGE ring (k-major: first matmul needs
                # xt k0 only); weights on the other ring m-chunked so each
                # m-group's full k-column lands as one transfer
                xts = xpool.tile([P, FK, cap], f32r, tag="xt")
                xt_r = xt[e].rearrange("(k p) n -> p k n", p=P)
                for k in range(FK):
                    nc.sync.dma_start(out=xts[:, k], in_=xt_r[:, k])
                w1s = w1pool.tile([P, FK, H], f32r, tag="w1")
                w1_r = w1[e].rearrange("(k p) m -> p k m", p=P)
                MW1 = H // 4
                for j in range(4):
                    nc.scalar.dma_start(
                        out=w1s[:, :, j * MW1:(j + 1) * MW1],
                        in_=w1_r[:, :, j * MW1:(j + 1) * MW1],
                    )
                w2s = w2pool.tile([P, HK, C], f32r, tag="w2")
                w2_r = w2[e].rearrange("(k p) m -> p k m", p=P)
                MW2 = C // 4
                for j in range(4):
                    nc.scalar.dma_start(
                        out=w2s[:, :, j * MW2:(j + 1) * MW2],
                        in_=w2_r[:, :, j * MW2:(j + 1) * MW2],
                    )

                a1s = apool.tile([P, HK, cap], f32r, tag="a1")
                yts = ypool.tile([P, CK, cap], f32, tag="yt")
                yt_r = yt[e].rearrange("(k p) n -> p k n", p=P)

                # all layer-1 chunks first, then all layer-2 chunks: PE has
                # ready work across the L1->L2 boundary and the e->e+1 seam
                ev = 0
                for n0, nsz in chunks:
                    # layer 1: A1T[m] = relu(sum_k W1[k,m]^T @ XgT[k] + b1[m])
                    for m in range(HK):
                        ps = pp.tile([P, 512], f32, tag="ps")
                        for k in range(FK):
                            nc.tensor.matmul(
                                ps[:, :nsz],
                                w1s[:, k, m * P:(m + 1) * P],
                                xts[:, k, n0:n0 + nsz],
                                start=(k == 0),
                                stop=(k == FK - 1),
                            )
                        evict(ev, a1s[:, m, n0:n0 + nsz], ps[:, :nsz],
                              b1s[:, e, m:m + 1], relu=True)
                        ev += 1
                for n0, nsz in chunks:
                    # layer 2: YT[m] = sum_k W2[k,m]^T @ A1T[k] + b2[m]
                    for m in range(CK):
                        ps = pp.tile([P, 512], f32, tag="ps")
                        for k in range(HK):
                            nc.tensor.matmul(
                                ps[:, :nsz],
                                w2s[:, k, m * P:(m + 1) * P],
                                a1s[:, k, n0:n0 + nsz],
                                start=(k == 0),
                                stop=(k == HK - 1),
                            )
                        evict(ev, yts[:, m, n0:n0 + nsz], ps[:, :nsz],
                              b2s[:, e, m:m + 1], relu=False)
                        ev += 1
                        if n0 + nsz == cap:
                            # whole row of yts done -> stream it out
                            nc.sync.dma_start(out=yt_r[:, m], in_=yts[:, m])

    nc.compile()
    return nc


def _chunks(cap):
    n = (cap + 511) // 512
    base, rem = divmod(cap, n)
    out = []
    off = 0
    for i in range(n):
        sz = base + (1 if i < rem else 0)
        out.append((off, sz))
        off += sz
    return out


# ---------------------------------------------------------------------------
# entry point
# ---------------------------------------------------------------------------
def kernel(h, W_mu, b_mu, W_logvar, b_logvar, W1, b1, W2, b2):
    from concourse.bass_utils import run_bass_kernel_spmd

    h = np.ascontiguousarray(np.asarray(h, dtype=np.float32))
    W1 = np.asarray(W1, dtype=np.float32)
    b1 = np.asarray(b1, dtype=np.float32)
    W2 = np.asarray(W2, dtype=np.float32)
    b2 = np.asarray(b2, dtype=np.float32)

    B, F = h.shape
    E, _, H = W1.shape
    C = W2.shape[2]
    assert E % NCORES == 0
    epc = E // NCORES
    FK, HK, CK = F // P, H // P, C // P

    topk_idx, topk_w = _routing(
        np.asarray(h), np.asarray(W_mu), np.asarray(b_mu),
        np.asarray(W_logvar), np.asarray(b_logvar)
    )

    # per-expert token lists
    toks, poss = [], []
    for e in range(E):
        tok, pos = np.nonzero(topk_idx == e)
        toks.append(tok)
        poss.append(pos)
    maxcnt = max(len(t) for t in toks)
    cap = max(64, -(-maxcnt // 32) * 32)
    chunks = _chunks(cap)

    # gather/dispatch: XgT per expert, padded to cap
    xt = np.zeros((NCORES, epc, F, cap), np.float32)
    for e in range(E):
        cnt = len(toks[e])
        xt[e // epc, e % epc, :, :cnt] = h[toks[e]].T

    # biases pre-swizzled to [P, epc, K] so the device DMA is contiguous
    b1_sw = np.ascontiguousarray(
        b1.reshape(E, HK, P).transpose(2, 0, 1).reshape(P, NCORES, epc, HK)
        .transpose(1, 0, 2, 3)
    )  # [NCORES, P, epc, HK]
    b2_sw = np.ascontiguousarray(
        b2.reshape(E, CK, P).transpose(2, 0, 1).reshape(P, NCORES, epc, CK)
        .transpose(1, 0, 2, 3)
    )

    nc = _build_kernel(epc, F, H, C, cap, chunks)

    in_maps = []
    for c in range(NCORES):
        lo, hi = c * epc, (c + 1) * epc
        in_maps.append({
            "xt": xt[c],
            "w1": W1[lo:hi],
            "w2": W2[lo:hi],
            "b1": b1_sw[c],
            "b2": b2_sw[c],
        })

    trace = bool(os.environ.get("MOE_KERNEL_TRACE"))
    res = run_bass_kernel_spmd(nc, in_maps, list(range(NCORES)), trace=trace)
    global LAST_RESULTS
    LAST_RESULTS = res

    # combine: scatter-add weighted expert outputs
    out = np.zeros((B, C), np.float32)
    for e in range(E):
        cnt = len(toks[e])
        yte = res.results[e // epc]["yt"][e % epc]  # [C, cap]
        out[toks[e]] += (
            topk_w[toks[e], poss[e]].astype(np.float32)[:, None]
            * yte[:, :cnt].T
        )
    return out


LAST_RESULTS = None


# revision 6
# speedup vs baseline: 1.1439x; 1.1308x over previous
"""Bayesian-router MoE kernel for 8 Trainium2 NeuronCores.

Strategy (expert-parallel, per sharding hint):
  - Router moments / top-k / combine weights: tiny (B*F*E ~ 17 MFLOP), computed
    on host in float64 (min rank4/rank5 score gap is ~1.7e-4, far above fp32
    noise, so expert selection is stable vs the fp32 reference).
  - Token dispatch: host gathers each expert's routed tokens into a padded,
    transposed buffer XgT [F, CAP] (the host-side equivalent of the
    all-to-all; full I/O contract means shard/unshard happens on host).
    Experts are sorted by token count: the 8 largest go to slot 0 (cap0),
    the 8 smallest to slot 1 (cap1 <= cap0), one of each per core, so the
    SPMD program wastes less padding compute.
  - Device: each of the 8 cores runs its 2-expert MLP on gathered tokens,
    entirely in transposed form (A1T = relu(W1^T XgT + b1), YT = W2^T A1T + b2)
    so no on-device transposes are needed; weights stream as lhsT directly.
    Matmuls run as float32r (full fp32 storage; reduced-precision multiply at
    1 cycle/row) -- measured ~4x faster than plain fp32 matmul.
  - Combine: host scatter-adds w[t,e] * Y_e rows into the output (the
    cross-device reduction of the unshard step).
"""

import os
import numpy as np

NCORES = 8
P = 128
TOP_K = 4


# ---------------------------------------------------------------------------
# host-side routing (matches reference math; float64 for stable ordering)
# ---------------------------------------------------------------------------
def _routing(h, W_mu, b_mu, W_logvar, b_logvar):
    h64 = h.astype(np.float64)
    mu = h64 @ W_mu.T.astype(np.float64) + b_mu.astype(np.float64)
    var = (h64 * h64) @ np.exp(W_logvar.astype(np.float64)).T + np.exp(
        b_logvar.astype(np.float64)
    )
    var = np.maximum(var, 1e-12)
    tilde = mu / np.sqrt(1.0 + (np.pi / 8.0) * var)
    t = tilde - tilde.max(axis=1, keepdims=True)
    ex = np.exp(t)
    probs = ex / ex.sum(axis=1, keepdims=True)
    idx = np.argsort(-tilde, axis=1, kind="stable")[:, :TOP_K]
    w = np.take_along_axis(probs, idx, axis=1)
    w = w / np.maximum(w.sum(axis=1, keepdims=True), 1e-12)
    return idx, w


def _chunks(cap):
    n = (cap + 511) // 512
    base, rem = divmod(cap, n)
    out = []
    off = 0
    for i in range(n):
        sz = base + (1 if i < rem else 0)
        out.append((off, sz))
        off += sz
    return out


# ---------------------------------------------------------------------------
# device kernel: 2-expert MLP on pre-gathered transposed tokens
# ---------------------------------------------------------------------------
def _build_kernel(F, H, C, caps):
    import concourse.mybir as mybir
    import concourse.tile as tile
    from concourse import bacc

    f32 = mybir.dt.float32
    f32r = mybir.dt.float32r
    FK, HK, CK = F // P, H // P, C // P
    nslots = len(caps)

    nc = bacc.Bacc("TRN2", target_bir_lowering=False, debug=False,
                   num_devices=NCORES)

    # inputs feeding fp32r matmuls are declared float32r end-to-end (same
    # bytes as fp32) so plain HWDGE DMAs satisfy the BIR fp32r-rounding rule
    xts_d = [nc.dram_tensor(f"xt{s}", [F, caps[s]], f32r, kind="ExternalInput")
             for s in range(nslots)]
    yts_d = [nc.dram_tensor(f"yt{s}", [C, caps[s]], f32, kind="ExternalOutput")
             for s in range(nslots)]
    w1 = nc.dram_tensor("w1", [nslots, F, H], f32r, kind="ExternalInput")
    w2 = nc.dram_tensor("w2", [nslots, H, C], f32r, kind="ExternalInput")
    b1 = nc.dram_tensor("b1", [P, nslots, HK], f32, kind="ExternalInput")
    b2 = nc.dram_tensor("b2", [P, nslots, CK], f32, kind="ExternalInput")

    with tile.TileContext(nc) as tc:
        with (
            tc.tile_pool(name="consts", bufs=1) as consts,
            tc.tile_pool(name="w1pool", bufs=2) as w1pool,
            tc.tile_pool(name="w2pool", bufs=2) as w2pool,
            tc.tile_pool(name="xpool", bufs=2) as xpool,
            tc.tile_pool(name="apool", bufs=2) as apool,
            tc.tile_pool(name="ypool", bufs=2) as ypool,
            tc.tile_pool(name="psum", bufs=6, space="PSUM") as pp,
        ):
            b1s = consts.tile([P, nslots, HK], f32)
            nc.gpsimd.dma_start(out=b1s[:], in_=b1[:])
            b2s = consts.tile([P, nslots, CK], f32)
            nc.gpsimd.dma_start(out=b2s[:], in_=b2[:])

            add, amax = mybir.AluOpType.add, mybir.AluOpType.max

            def evict(i, dst, src, bias, relu):
                # alternate PSUM evictions between Scalar(ACT) and Vector(DVE)
                # so neither engine falls behind the matmul stream
                if i % 2 == 0:
                    nc.scalar.activation(
                        dst, src,
                        mybir.ActivationFunctionType.Relu if relu
                        else mybir.ActivationFunctionType.Identity,
                        bias=bias,
                    )
                elif relu:
                    nc.vector.tensor_scalar(dst, src, bias, 0.0, add, amax)
                else:
                    nc.vector.tensor_scalar_add(dst, src, bias)

            for s in range(nslots):
                cap = caps[s]
                chunks = _chunks(cap)
                # token chunks on one HWDGE ring (k-chunked); weights on the
                # other ring m-chunked so each m-group's full k-column lands
                # as one transfer, just ahead of the matmuls that need it
                xts = xpool.tile([P, FK, cap], f32r, tag=f"xt{s}")
                xt_r = xts_d[s].rearrange("(k p) n -> p k n", p=P)
                for k in range(FK):
                    nc.sync.dma_start(out=xts[:, k], in_=xt_r[:, k])
                w1s = w1pool.tile([P, FK, H], f32r, tag="w1")
                w1_r = w1[s].rearrange("(k p) m -> p k m", p=P)
                MW1 = H // 4
                for j in range(4):
                    nc.scalar.dma_start(
                        out=w1s[:, :, j * MW1:(j + 1) * MW1],
                        in_=w1_r[:, :, j * MW1:(j + 1) * MW1],
                    )
                w2s = w2pool.tile([P, HK, C], f32r, tag="w2")
                w2_r = w2[s].rearrange("(k p) m -> p k m", p=P)
                MW2 = C // 4
                for j in range(4):
                    nc.scalar.dma_start(
                        out=w2s[:, :, j * MW2:(j + 1) * MW2],
                        in_=w2_r[:, :, j * MW2:(j + 1) * MW2],
                    )

                a1s = apool.tile([P, HK, cap], f32r, tag="a1")
                ysb = ypool.tile([P, CK, cap], f32, tag="yt")
                yt_r = yts_d[s].rearrange("(k p) n -> p k n", p=P)

                # all layer-1 m-groups first, then all layer-2: PE has ready
                # work across the L1->L2 boundary and the slot seam
                ev = 0
                for m in range(HK):
                    for n0, nsz in chunks:
                        ps = pp.tile([P, 512], f32, tag="ps")
                        for k in range(FK):
                            nc.tensor.matmul(
                                ps[:, :nsz],
                                w1s[:, k, m * P:(m + 1) * P],
                                xts[:, k, n0:n0 + nsz],
                                start=(k == 0),
                                stop=(k == FK - 1),
                            )
                        evict(ev, a1s[:, m, n0:n0 + nsz], ps[:, :nsz],
                              b1s[:, s, m:m + 1], relu=True)
                        ev += 1
                for m in range(CK):
                    for n0, nsz in chunks:
                        ps = pp.tile([P, 512], f32, tag="ps")
                        for k in range(HK):
                            nc.tensor.matmul(
                                ps[:, :nsz],
                                w2s[:, k, m * P:(m + 1) * P],
                                a1s[:, k, n0:n0 + nsz],
                                start=(k == 0),
                                stop=(k == HK - 1),
                            )
                        evict(ev, ysb[:, m, n0:n0 + nsz], ps[:, :nsz],
                              b2s[:, s, m:m + 1], relu=False)
                        ev += 1
                        if n0 + nsz == cap:
                            # whole row of ysb done -> stream it out
                            nc.sync.dma_start(out=yt_r[:, m], in_=ysb[:, m])

    nc.compile()
    return nc


# ---------------------------------------------------------------------------
# entry point
# ---------------------------------------------------------------------------
def kernel(h, W_mu, b_mu, W_logvar, b_logvar, W1, b1, W2, b2):
    from concourse.bass_utils import run_bass_kernel_spmd

    h = np.ascontiguousarray(np.asarray(h, dtype=np.float32))
    W1 = np.asarray(W1, dtype=np.float32)
    b1 = np.asarray(b1, dtype=np.float32)
    W2 = np.asarray(W2, dtype=np.float32)
    b2 = np.asarray(b2, dtype=np.float32)

    B, F = h.shape
    E, _, H = W1.shape
    C = W2.shape[2]
    assert E % NCORES == 0
    nslots = E // NCORES
    FK, HK, CK = F // P, H // P, C // P

    topk_idx, topk_w = _routing(
        np.asarray(h), np.asarray(W_mu), np.asarray(b_mu),
        np.asarray(W_logvar), np.asarray(b_logvar)
    )

    # per-expert token lists; sort experts by count so each slot's capacity
    # is the max within that slot (slot 0 = busiest experts)
    toks, poss = [], []
    counts = np.zeros(E, np.int64)
    for e in range(E):
        tok, pos = np.nonzero(topk_idx == e)
        toks.append(tok)
        poss.append(pos)
        counts[e] = len(tok)
    perm = np.argsort(-counts, kind="stable")
    caps = []
    for s in range(nslots):
        grp = perm[s * NCORES:(s + 1) * NCORES]
        caps.append(max(64, int(-(-counts[grp].max() // 32) * 32)))

    # gather/dispatch: XgT per expert, padded to its slot's cap
    xt = [np.zeros((NCORES, F, caps[s]), np.float32) for s in range(nslots)]
    w1_in = np.empty((NCORES, nslots, F, H), np.float32)
    w2_in = np.empty((NCORES, nslots, H, C), np.float32)
    b1_in = np.empty((NCORES, P, nslots, HK), np.float32)
    b2_in = np.empty((NCORES, P, nslots, CK), np.float32)
    for i, e in enumerate(perm):
        s, c = divmod(i, NCORES)
        xt[s][c, :, :counts[e]] = h[toks[e]].T
        w1_in[c, s] = W1[e]
        w2_in[c, s] = W2[e]
        b1_in[c, :, s, :] = b1[e].reshape(HK, P).T
        b2_in[c, :, s, :] = b2[e].reshape(CK, P).T

    nc = _build_kernel(F, H, C, caps)

    in_maps = []
    for c in range(NCORES):
        m = {"w1": w1_in[c], "w2": w2_in[c], "b1": b1_in[c], "b2": b2_in[c]}
        for s in range(nslots):
            m[f"xt{s}"] = xt[s][c]
        in_maps.append(m)

    trace = bool(os.environ.get("MOE_KERNEL_TRACE"))
    res = run_bass_kernel_spmd(nc, in_maps, list(range(NCORES)), trace=trace)
    global LAST_RESULTS
    LAST_RESULTS = res

    # combine: scatter-add weighted expert outputs
    out = np.zeros((B, C), np.float32)
    for i, e in enumerate(perm):
        s, c = divmod(i, NCORES)
        cnt = counts[e]
        yte = res.results[c][f"yt{s}"]  # [C, cap_s]
        out[toks[e]] += (
            topk_w[toks[e], poss[e]].astype(np.float32)[:, None]
            * yte[:, :cnt].T
        )
    return out


LAST_RESULTS = None


# revision 7
# speedup vs baseline: 1.1506x; 1.0058x over previous
"""Bayesian-router MoE kernel for 8 Trainium2 NeuronCores.

Strategy (expert-parallel, per sharding hint):
  - Router moments / top-k / combine weights: tiny (B*F*E ~ 17 MFLOP), computed
    on host in float64 (min rank4/rank5 score gap is ~1.7e-4, far above fp32
    noise, so expert selection is stable vs the fp32 reference).
  - Token dispatch: host gathers each expert's routed tokens into a padded,
    transposed buffer XgT [F, CAP] (the host-side equivalent of the
    all-to-all; full I/O contract means shard/unshard happens on host).
    Experts are sorted by token count: the 8 largest go to slot 0 (cap0),
    the 8 smallest to slot 1 (cap1 <= cap0), one of each per core, so the
    SPMD program wastes less padding compute.
  - Device: each of the 8 cores runs its 2-expert MLP on gathered tokens,
    entirely in transposed form (A1T = relu(W1^T XgT + b1), YT = W2^T A1T + b2)
    so no on-device transposes are needed; weights stream as lhsT directly.
    Matmuls run as float32r (full fp32 storage; reduced-precision multiply at
    1 cycle/row) -- measured ~4x faster than plain fp32 matmul.
  - Combine: host scatter-adds w[t,e] * Y_e rows into the output (the
    cross-device reduction of the unshard step).
"""

import os
import numpy as np

NCORES = 8
P = 128
TOP_K = 4


# ---------------------------------------------------------------------------
# host-side routing (matches reference math; float64 for stable ordering)
# ---------------------------------------------------------------------------
def _routing(h, W_mu, b_mu, W_logvar, b_logvar):
    h64 = h.astype(np.float64)
    mu = h64 @ W_mu.T.astype(np.float64) + b_mu.astype(np.float64)
    var = (h64 * h64) @ np.exp(W_logvar.astype(np.float64)).T + np.exp(
        b_logvar.astype(np.float64)
    )
    var = np.maximum(var, 1e-12)
    tilde = mu / np.sqrt(1.0 + (np.pi / 8.0) * var)
    t = tilde - tilde.max(axis=1, keepdims=True)
    ex = np.exp(t)
    probs = ex / ex.sum(axis=1, keepdims=True)
    idx = np.argsort(-tilde, axis=1, kind="stable")[:, :TOP_K]
    w = np.take_along_axis(probs, idx, axis=1)
    w = w / np.maximum(w.sum(axis=1, keepdims=True), 1e-12)
    return idx, w


def _chunks(cap):
    n = (cap + 511) // 512
    base, rem = divmod(cap, n)
    out = []
    off = 0
    for i in range(n):
        sz = base + (1 if i < rem else 0)
        out.append((off, sz))
        off += sz
    return out


# ---------------------------------------------------------------------------
# device kernel: 2-expert MLP on pre-gathered transposed tokens
# ---------------------------------------------------------------------------
def _build_kernel(F, H, C, caps):
    import concourse.mybir as mybir
    import concourse.tile as tile
    from concourse import bacc

    f32 = mybir.dt.float32
    f32r = mybir.dt.float32r
    FK, HK, CK = F // P, H // P, C // P
    nslots = len(caps)

    nc = bacc.Bacc("TRN2", target_bir_lowering=False, debug=False,
                   num_devices=NCORES)

    # inputs feeding fp32r matmuls are declared float32r end-to-end (same
    # bytes as fp32) so plain HWDGE DMAs satisfy the BIR fp32r-rounding rule
    xts_d = [nc.dram_tensor(f"xt{s}", [F, caps[s]], f32r, kind="ExternalInput")
             for s in range(nslots)]
    yts_d = [nc.dram_tensor(f"yt{s}", [C, caps[s]], f32, kind="ExternalOutput")
             for s in range(nslots)]
    w1 = nc.dram_tensor("w1", [nslots, F, H], f32r, kind="ExternalInput")
    w2 = nc.dram_tensor("w2", [nslots, H, C], f32r, kind="ExternalInput")
    b1 = nc.dram_tensor("b1", [P, nslots, HK], f32, kind="ExternalInput")
    b2 = nc.dram_tensor("b2", [P, nslots, CK], f32, kind="ExternalInput")

    with tile.TileContext(nc) as tc:
        with (
            tc.tile_pool(name="consts", bufs=1) as consts,
            tc.tile_pool(name="w1pool", bufs=2) as w1pool,
            tc.tile_pool(name="w2pool", bufs=2) as w2pool,
            tc.tile_pool(name="xpool", bufs=2) as xpool,
            tc.tile_pool(name="apool", bufs=2) as apool,
            tc.tile_pool(name="ypool", bufs=2) as ypool,
            tc.tile_pool(name="psum", bufs=8, space="PSUM") as pp,
        ):
            b1s = consts.tile([P, nslots, HK], f32)
            nc.gpsimd.dma_start(out=b1s[:], in_=b1[:])
            b2s = consts.tile([P, nslots, CK], f32)
            nc.gpsimd.dma_start(out=b2s[:], in_=b2[:])

            add, amax = mybir.AluOpType.add, mybir.AluOpType.max

            def evict(i, dst, src, bias, relu):
                # alternate PSUM evictions between Scalar(ACT) and Vector(DVE)
                # so neither engine falls behind the matmul stream
                if i % 2 == 0:
                    nc.scalar.activation(
                        dst, src,
                        mybir.ActivationFunctionType.Relu if relu
                        else mybir.ActivationFunctionType.Identity,
                        bias=bias,
                    )
                elif relu:
                    nc.vector.tensor_scalar(dst, src, bias, 0.0, add, amax)
                else:
                    nc.vector.tensor_scalar_add(dst, src, bias)

            for s in range(nslots):
                cap = caps[s]
                chunks = _chunks(cap)
                # token chunks on one HWDGE ring (k-chunked); weights on the
                # other ring m-chunked so each m-group's full k-column lands
                # as one transfer, just ahead of the matmuls that need it
                xts = xpool.tile([P, FK, cap], f32r, tag=f"xt{s}")
                xt_r = xts_d[s].rearrange("(k p) n -> p k n", p=P)
                for k in range(FK):
                    nc.sync.dma_start(out=xts[:, k], in_=xt_r[:, k])
                w1s = w1pool.tile([P, FK, H], f32r, tag="w1")
                w1_r = w1[s].rearrange("(k p) m -> p k m", p=P)
                MW1 = H // 4
                for j in range(4):
                    nc.scalar.dma_start(
                        out=w1s[:, :, j * MW1:(j + 1) * MW1],
                        in_=w1_r[:, :, j * MW1:(j + 1) * MW1],
                    )
                w2s = w2pool.tile([P, HK, C], f32r, tag="w2")
                w2_r = w2[s].rearrange("(k p) m -> p k m", p=P)
                MW2 = C // 4
                for j in range(4):
                    nc.scalar.dma_start(
                        out=w2s[:, :, j * MW2:(j + 1) * MW2],
                        in_=w2_r[:, :, j * MW2:(j + 1) * MW2],
                    )

                a1s = apool.tile([P, HK, cap], f32r, tag="a1")
                ysb = ypool.tile([P, CK, cap], f32, tag="yt")
                yt_r = yts_d[s].rearrange("(k p) n -> p k n", p=P)

                # all layer-1 m-groups first, then all layer-2: PE has ready
                # work across the L1->L2 boundary and the slot seam
                ev = 0
                for m in range(HK):
                    for n0, nsz in chunks:
                        ps = pp.tile([P, 512], f32, tag="ps")
                        for k in range(FK):
                            nc.tensor.matmul(
                                ps[:, :nsz],
                                w1s[:, k, m * P:(m + 1) * P],
                                xts[:, k, n0:n0 + nsz],
                                start=(k == 0),
                                stop=(k == FK - 1),
                            )
                        evict(ev, a1s[:, m, n0:n0 + nsz], ps[:, :nsz],
                              b1s[:, s, m:m + 1], relu=True)
                        ev += 1
                for m in range(CK):
                    for n0, nsz in chunks:
                        ps = pp.tile([P, 512], f32, tag="ps")
                        for k in range(HK):
                            nc.tensor.matmul(
                                ps[:, :nsz],
                                w2s[:, k, m * P:(m + 1) * P],
                                a1s[:, k, n0:n0 + nsz],
                                start=(k == 0),
                                stop=(k == HK - 1),
                            )
                        evict(ev, ysb[:, m, n0:n0 + nsz], ps[:, :nsz],
                              b2s[:, s, m:m + 1], relu=False)
                        ev += 1
                        if n0 + nsz == cap:
                            # whole row of ysb done -> stream it out
                            nc.sync.dma_start(out=yt_r[:, m], in_=ysb[:, m])

    nc.compile()
    return nc


# ---------------------------------------------------------------------------
# entry point
# ---------------------------------------------------------------------------
def kernel(h, W_mu, b_mu, W_logvar, b_logvar, W1, b1, W2, b2):
    from concourse.bass_utils import run_bass_kernel_spmd

    h = np.ascontiguousarray(np.asarray(h, dtype=np.float32))
    W1 = np.asarray(W1, dtype=np.float32)
    b1 = np.asarray(b1, dtype=np.float32)
    W2 = np.asarray(W2, dtype=np.float32)
    b2 = np.asarray(b2, dtype=np.float32)

    B, F = h.shape
    E, _, H = W1.shape
    C = W2.shape[2]
    assert E % NCORES == 0
    nslots = E // NCORES
    FK, HK, CK = F // P, H // P, C // P

    topk_idx, topk_w = _routing(
        np.asarray(h), np.asarray(W_mu), np.asarray(b_mu),
        np.asarray(W_logvar), np.asarray(b_logvar)
    )

    # per-expert token lists; sort experts by count so each slot's capacity
    # is the max within that slot (slot 0 = busiest experts)
    toks, poss = [], []
    counts = np.zeros(E, np.int64)
    for e in range(E):
        tok, pos = np.nonzero(topk_idx == e)
        toks.append(tok)
        poss.append(pos)
        counts[e] = len(tok)
    perm = np.argsort(-counts, kind="stable")
    caps = []
    for s in range(nslots):
        grp = perm[s * NCORES:(s + 1) * NCORES]
        caps.append(max(64, int(-(-counts[grp].max() // 32) * 32)))

    # gather/dispatch: XgT per expert, padded to its slot's cap
    xt = [np.zeros((NCORES, F, caps[s]), np.float32) for s in range(nslots)]
    w1_in = np.empty((NCORES, nslots, F, H), np.float32)
    w2_in = np.empty((NCORES, nslots, H, C), np.float32)
    b1_in = np.empty((NCORES, P, nslots, HK), np.float32)
    b2_in = np.empty((NCORES, P, nslots, CK), np.float32)
    for i, e in enumerate(perm):
        s, c = divmod(i, NCORES)
        xt[s][c, :, :counts[e]] = h[toks[e]].T
        w1_in[c, s] = W1[e]
        w2_in[c, s] = W2[e]
        b1_in[c, :, s, :] = b1[e].reshape(HK, P).T
        b2_in[c, :, s, :] = b2[e].reshape(CK, P).T

    nc = _build_kernel(F, H, C, caps)

    in_maps = []
    for c in range(NCORES):
        m = {"w1": w1_in[c], "w2": w2_in[c], "b1": b1_in[c], "b2": b2_in[c]}
        for s in range(nslots):
            m[f"xt{s}"] = xt[s][c]
        in_maps.append(m)

    trace = bool(os.environ.get("MOE_KERNEL_TRACE"))
    res = run_bass_kernel_spmd(nc, in_maps, list(range(NCORES)), trace=trace)
    global LAST_RESULTS
    LAST_RESULTS = res

    # combine: scatter-add weighted expert outputs
    out = np.zeros((B, C), np.float32)
    for i, e in enumerate(perm):
        s, c = divmod(i, NCORES)
        cnt = counts[e]
        yte = res.results[c][f"yt{s}"]  # [C, cap_s]
        out[toks[e]] += (
            topk_w[toks[e], poss[e]].astype(np.float32)[:, None]
            * yte[:, :cnt].T
        )
    return out


LAST_RESULTS = None
